# revision 3
# baseline (speedup 1.0000x reference)
"""DeepseekV3 top-k router kernel for Trainium2 (Bass/Tile), 8-core SPMD.

v4: supertile-batched engine passes + packed-key weight extraction.

Per supertile (B=8 tiles of [128 tokens x 256 experts], token-per-partition):
  ACT : s = Sigmoid(L)                [P,2048] one op
  ACT : k2 = s*16384 + 2^23           (magic round -> integer s_q = round(s*16384))
  ACT : k3 = k2*256 - 2^31            ( = s_q*256, exact )
  Pool: s4c  = s + bias               [P,2048]
  Pool: keys = k3 + iota256           [P,2048]  (key = s_q*256 + expert_id)
  DVE : per-group top-8 via 8x max8 per tile -> cand g8 [P,64] per tile
  DVE : group sort via max8 on [P,8] scores; mask01 = gs > 5th largest
  Pool: m64 = g8 * mask01
  DVE : v8 = max8(m64); i8 = find_index8(v8, s4c)
  DVE : marked = match_replace(v8, s4c, 2^30)
  DVE : all2 = marked*2^-7 + keys     [P,2048] one stt
        (selected slots become 2^23+key, others stay < 2^22+1)
  DVE : kv8 = max8(all2 slice)  -> packed (s_q, id) of the selected 8
  smalls: unpack s_q/id, realign to i8 order, normalize 2.5/dens
"""
import numpy as np

import concourse.bass as bass
import concourse.bacc as bacc
import concourse.mybir as mybir
from concourse.tile import TileContext
from concourse.bass_utils import run_bass_kernel_spmd

F32 = mybir.dt.float32
U32 = mybir.dt.uint32
I32 = mybir.dt.int32

T_FULL = 131072
E = 256
N_CORES = 8
T_CORE = T_FULL // N_CORES      # 16384
P = 128
N_TILES = T_CORE // P           # 128
B = 8                           # tiles per supertile
N_SUPER = N_TILES // B          # 16
G = 8
EG = E // G                     # 32
FLAG = float(2.0 ** 30)         # match_replace marker
MAGIC = 12582912.0              # 1.5 * 2^23

LAST_EXEC_NS = None
LAST_RESULTS = None


def _build(nc: bass.Bass):
    x_d = nc.dram_tensor("logits", [T_CORE, E], F32, kind="ExternalInput")
    b_d = nc.dram_tensor("bias", [1, E], F32, kind="ExternalInput")
    idx_d = nc.dram_tensor("idx_out", [T_CORE, 8], U32, kind="ExternalOutput")
    w_d = nc.dram_tensor("w_out", [T_CORE, 8], F32, kind="ExternalOutput")

    AX = mybir.AxisListType.X
    OP = mybir.AluOpType
    ACTF = mybir.ActivationFunctionType
    BE = B * E  # 2048

    with TileContext(nc) as tc:
        with tc.tile_pool(name="const", bufs=1) as cpool, \
             tc.tile_pool(name="io", bufs=2) as iopool, \
             tc.tile_pool(name="keep", bufs=2) as kpool, \
             tc.tile_pool(name="slot", bufs=2) as slpool:

            biasb = cpool.tile([P, E], F32)
            nc.gpsimd.dma_start(out=biasb[:], in_=b_d[:, :].to_broadcast((P, E)))
            iotai = cpool.tile([P, E], I32)
            nc.gpsimd.iota(iotai[:], pattern=[[1, E]], base=0,
                           channel_multiplier=0)
            iotaf = cpool.tile([P, E], F32)
            nc.vector.tensor_copy(out=iotaf[:], in_=iotai[:])

            for sp_i in range(N_SUPER):
                ss = kpool.tile([P, BE], F32, tag="ss")
                k2s = kpool.tile([P, BE], F32, tag="k2s")
                k3s = kpool.tile([P, BE], F32, tag="k3s")
                s4cs = kpool.tile([P, BE], F32, tag="s4cs")
                keys = kpool.tile([P, BE], F32, tag="keys")
                markeds = kpool.tile([P, BE], F32, tag="markeds")
                all2 = kpool.tile([P, BE], F32, tag="all2")

                g8s = slpool.tile([P, B * 64], F32, tag="g8s")
                m64s = slpool.tile([P, B * 64], F32, tag="m64s")
                gss = slpool.tile([P, B * G], F32, tag="gss")
                gsorts = slpool.tile([P, B * G], F32, tag="gsorts")
                mask01 = slpool.tile([P, B * G], F32, tag="mask01")
                v8s = slpool.tile([P, B * 8], F32, tag="v8s")
                i8s = slpool.tile([P, B * 8], U32, tag="i8s")
                i8f = slpool.tile([P, B * 8], F32, tag="i8f")
                i8q = slpool.tile([P, B * 8], F32, tag="i8q")
                kv8s = slpool.tile([P, B * 8], F32, tag="kv8s")
                kvm23 = slpool.tile([P, B * 8], F32, tag="kvm23")
                t1s = slpool.tile([P, B * 8], F32, tag="t1s")
                vqs = slpool.tile([P, B * 8], F32, tag="vqs")
                w8sq = slpool.tile([P, B * 8], F32, tag="w8sq")
                diffq = slpool.tile([P, B * 8], F32, tag="diffq")
                dens = slpool.tile([P, B], F32, tag="dens")
                rdens = slpool.tile([P, B], F32, tag="rdens")
                eqms = slpool.tile([P, B * 64], F32, tag="eqms")
                wms = slpool.tile([P, B * 64], F32, tag="wms")
                w8s = slpool.tile([P, B * 8], F32, tag="w8s")
                wouts = slpool.tile([P, B * 8], F32, tag="wouts")

                srow = sp_i * B * P
                Ls = iopool.tile([P, BE], F32, tag="L")
                nc.sync.dma_start(
                    out=Ls[:],
                    in_=x_d[srow:srow + B * P, :].rearrange(
                        "(p x) e -> p (x e)", p=P))

                # ---- batched full-width passes ----
                nc.scalar.activation(ss[:], Ls[:], ACTF.Sigmoid)
                nc.scalar.activation(k2s[:], ss[:], ACTF.Copy,
                                     scale=16384.0, bias=float(2 ** 23))
                nc.scalar.activation(k3s[:], k2s[:], ACTF.Copy,
                                     scale=256.0, bias=float(-2 ** 31))
                nc.gpsimd.tensor_tensor(
                    s4cs[:].rearrange("p (b e) -> p b e", b=B),
                    ss[:].rearrange("p (b e) -> p b e", b=B),
                    biasb[:].unsqueeze(1).broadcast_to([P, B, E]), op=OP.add)
                nc.gpsimd.tensor_tensor(
                    keys[:].rearrange("p (b e) -> p b e", b=B),
                    k3s[:].rearrange("p (b e) -> p b e", b=B),
                    iotaf[:].unsqueeze(1).broadcast_to([P, B, E]), op=OP.add)

                # ---- per-group top-8 candidates ----
                for b in range(B):
                    for g in range(G):
                        nc.vector.max(
                            out=g8s[:, b * 64 + g * 8: b * 64 + g * 8 + 8],
                            in_=s4cs[:, b * E + g * EG: b * E + (g + 1) * EG])

                # ---- group ranking ----
                g84 = g8s[:].rearrange("p (b g k) -> p b g k", b=B, g=G)
                nc.gpsimd.tensor_tensor(
                    gss[:].rearrange("p (b g) -> p b g", b=B),
                    g84[:, :, :, 0], g84[:, :, :, 1], op=OP.add)
                for b in range(B):
                    nc.vector.max(out=gsorts[:, b * G:(b + 1) * G],
                                  in_=gss[:, b * G:(b + 1) * G])
                # mask01 = (gs > 5th largest) -> exactly top-4 groups
                nc.vector.tensor_tensor(
                    mask01[:].rearrange("p (b g) -> p b g", b=B),
                    gss[:].rearrange("p (b g) -> p b g", b=B),
                    gsorts[:].rearrange("p (b g) -> p b g", b=B)[:, :, 4:5]
                        .broadcast_to([P, B, G]),
                    op=OP.is_gt)
                nc.gpsimd.tensor_tensor(
                    m64s[:].rearrange("p (b g k) -> p b g k", b=B, g=G),
                    g84,
                    mask01[:].rearrange("p (b g) -> p b g", b=B)
                        .unsqueeze(3).broadcast_to([P, B, G, 8]),
                    op=OP.mult)

                # ---- selection + marking per tile ----
                for b in range(B):
                    v8 = v8s[:, b * 8:(b + 1) * 8]
                    nc.vector.max(out=v8, in_=m64s[:, b * 64:(b + 1) * 64])
                    nc.vector.max_index(i8s[:, b * 8:(b + 1) * 8], v8,
                                        s4cs[:, b * E:(b + 1) * E])
                    nc.vector.match_replace(
                        out=markeds[:, b * E:(b + 1) * E], in_to_replace=v8,
                        in_values=s4cs[:, b * E:(b + 1) * E], imm_value=FLAG)

                # ---- packed-key extraction (batched stt + per-tile max8) ----
                nc.vector.scalar_tensor_tensor(
                    all2[:], markeds[:], float(2.0 ** -7), keys[:],
                    op0=OP.mult, op1=OP.add)
                for b in range(B):
                    nc.vector.max(out=kv8s[:, b * 8:(b + 1) * 8],
                                  in_=all2[:, b * E:(b + 1) * E])

                # ---- batched unpack ----
                # kvm23 = (kv8 - 2^23) * 2^-8 = s_q + id/256 (exact)
                nc.vector.tensor_scalar(kvm23[:], kv8s[:], float(-2 ** 23),
                                        float(2.0 ** -8), op0=OP.add,
                                        op1=OP.mult)
                # v = round-to-int via magic (no half-integer ties)
                nc.vector.tensor_scalar(t1s[:], kvm23[:], -0.498046875,
                                        MAGIC, op0=OP.add, op1=OP.add)
                nc.vector.tensor_scalar(vqs[:], t1s[:], -MAGIC, None,
                                        op0=OP.add)
                nc.vector.tensor_scalar(w8sq[:], vqs[:], float(1.0 / 16384.0),
                                        None, op0=OP.mult)
                # diffq = kvm23 - v = id/256 (exact)
                nc.vector.scalar_tensor_tensor(
                    diffq[:], vqs[:], -1.0, kvm23[:], op0=OP.mult, op1=OP.add)
                # i8q = i8 / 256 for exact equality match
                nc.vector.tensor_copy(out=i8f[:], in_=i8s[:])
                nc.vector.tensor_scalar(i8q[:], i8f[:], float(1.0 / 256.0),
                                        None, op0=OP.mult)

                # ---- realign to i8 order + normalize ----
                i8q3 = i8q[:].rearrange("p (b i) -> p b i", b=B)
                df3 = diffq[:].rearrange("p (b k) -> p b k", b=B)
                nc.vector.tensor_tensor(
                    eqms[:].rearrange("p (b i k) -> p b i k", b=B, i=8),
                    i8q3.unsqueeze(3).broadcast_to([P, B, 8, 8]),
                    df3.unsqueeze(2).broadcast_to([P, B, 8, 8]),
                    op=OP.is_equal)
                w3 = w8sq[:].rearrange("p (b k) -> p b k", b=B)
                nc.gpsimd.tensor_tensor(
                    wms[:].rearrange("p (b i k) -> p b i k", b=B, i=8),
                    eqms[:].rearrange("p (b i k) -> p b i k", b=B, i=8),
                    w3.unsqueeze(2).broadcast_to([P, B, 8, 8]),
                    op=OP.mult)
                nc.vector.tensor_reduce(
                    w8s[:], wms[:].rearrange("p (b i k) -> p b i k", b=B, i=8),
                    axis=AX, op=OP.add)
                nc.vector.tensor_reduce(
                    dens[:], w8sq[:].rearrange("p (b k) -> p b k", b=B),
                    axis=AX, op=OP.add)
                nc.vector.reciprocal(rdens[:], dens[:])
                rd3 = rdens[:].rearrange("p (b o) -> p b o", b=B)
                nc.vector.scalar_tensor_tensor(
                    wouts[:].rearrange("p (b i) -> p b i", b=B),
                    w8s[:].rearrange("p (b i) -> p b i", b=B), 2.5,
                    rd3.broadcast_to([P, B, 8]),
                    op0=OP.mult, op1=OP.mult)

                nc.sync.dma_start(
                    out=idx_d[srow:srow + B * P, :].rearrange(
                        "(p x) e -> p (x e)", p=P),
                    in_=i8s[:])
                nc.sync.dma_start(
                    out=w_d[srow:srow + B * P, :].rearrange(
                        "(p x) e -> p (x e)", p=P),
                    in_=wouts[:])
    return nc


_COMPILED_NC = None


def _get_nc():
    global _COMPILED_NC
    if _COMPILED_NC is None:
        nc = bacc.Bacc(None, target_bir_lowering=False, debug=False)
        _build(nc)
        nc.finalize()
        _COMPILED_NC = nc
    return _COMPILED_NC


def kernel(router_logits: np.ndarray, correction_bias: np.ndarray,
           trace: bool = False):
    global LAST_EXEC_NS, LAST_RESULTS
    x = np.ascontiguousarray(np.asarray(router_logits), dtype=np.float32)
    b = np.ascontiguousarray(np.asarray(correction_bias),
                             dtype=np.float32).reshape(1, E)
    assert x.shape == (T_FULL, E), x.shape

    nc = _get_nc()
    in_maps = [{"logits": x[c * T_CORE:(c + 1) * T_CORE], "bias": b}
               for c in range(N_CORES)]
    res = run_bass_kernel_spmd(nc, in_maps, core_ids=list(range(N_CORES)),
                               trace=trace)
    LAST_EXEC_NS = res.exec_time_ns
    LAST_RESULTS = res

    idx = np.concatenate([r["idx_out"] for r in res.results], axis=0)
    w = np.concatenate([r["w_out"] for r in res.results], axis=0)
    return idx.view(np.int32), w.astype(np.float32, copy=False)
